# revision 17
# baseline (speedup 1.0000x reference)
"""Trainium2 Bass kernel for the L2-normalized attention module.

Reference computation (per batch b):
    qkv = x @ w_qkv.T                        # [n, 3*dim]
    q,k,v per head h (d=64)                  # [n, d]
    dots = q @ k.T                           # [n, n]
    attn = dots / max(||dots_row||_2, eps) * g + b
    out_h = attn @ v                         # [n, d]
    final = concat_h(out_h) @ w_out.T + b_out

Key algebraic factorization used here: the score "nonlinearity" is only a
per-row scale r_i = 1/max(||dots_i||, eps), and ||dots_i||^2 = q_i^T (k^T k) q_i.
Therefore (with W = k^T v, G = k^T k):
    out_h^T = diag-free:  outT[:, i] = r_i * (W^T q_i),   r_i = rsqrt(q_i^T G q_i)
This removes the n x n score matrix entirely (8x fewer FLOPs) and keeps all
on-chip traffic tiny.  Since out is homogeneous degree-0 in q, wq is scaled
by 1/4 on the host (exact power of two) so the q'(Gq') products fit fp16.

Sharding: 8 cores = 2 batches x 4 head-groups (4 heads each).  Each core
computes its qkv slice, the factored attention for its 4 heads, and a partial
w_out projection; the host sums the 4 partials per batch (TP reduction) and
adds b_out.  norm_g is folded into w_out on the host; norm_b (zero in
practice) is handled by an exact host-side rank-1 correction.

Schedule: [warmup matmuls on memset tiles] -> [kv projection + Gram
accumulation for all chunks] -> [software-pipelined back half: q projection
of chunk c overlapped with the norm chain and w_out projection of chunk c-1].
Inputs split across both HWDGE rings (x on sync, weights on scalar).
"""

import numpy as np

from concourse import bacc
import concourse.mybir as mybir
import concourse.tile as tile
from concourse.bass_utils import run_bass_kernel_spmd

# Problem shape (hardcoded per contract)
B, N, DIM, H, D = 2, 2048, 1024, 16, 64
NCORES = 8
HPC = H // 4            # 4 heads per core
CH = 512                # sequence chunk (matmul moving free dim)
NCH = N // CH           # 4
KO = DIM // 128         # 8 contraction tiles for the projections
P = 128

F32 = mybir.dt.float32
F16 = mybir.dt.float16
MULT = mybir.AluOpType.mult
AFT = mybir.ActivationFunctionType
EPS2 = 6.25e-26         # (1e-12 / 4)^2: F.normalize eps with the q/4 scaling


def _build_bass():
    nc = bacc.Bacc("TRN2", target_bir_lowering=False, debug=False)

    # register the Rsqrt bias const (mirrors Bass's const registration)
    _eps_t = nc.alloc_sbuf_tensor("const-float32-epsr", [128, 1], F32)
    nc.gpsimd.memset(_eps_t.ap(), EPS2)
    nc.const_aps.aps[(F32, EPS2)] = _eps_t.ap()

    x_d = nc.dram_tensor("xt", [NCH, P, KO, CH], F16, kind="ExternalInput").ap()
    wq_d = nc.dram_tensor("wq", [P, KO, 256], F16, kind="ExternalInput").ap()
    wkv_d = nc.dram_tensor("wkv", [P, KO, 512], F16, kind="ExternalInput").ap()
    wo_d = nc.dram_tensor("wo", [P, 2, 1024], F16, kind="ExternalInput").ap()
    bo_d = nc.dram_tensor("bo", [P, 128], F16, kind="ExternalInput").ap()
    out_d = nc.dram_tensor("outT", [P, 8, N], F16, kind="ExternalOutput").ap()

    with tile.TileContext(nc) as tc:
        with (
            tc.tile_pool(name="w", bufs=1) as wpool,
            tc.tile_pool(name="big", bufs=1) as bigpool,
            tc.tile_pool(name="small", bufs=4) as smallpool,
            tc.tile_pool(name="stage", bufs=4) as stagepool,
            tc.tile_pool(name="ps4", bufs=4, space="PSUM") as ps4,
            tc.tile_pool(name="psA", bufs=2, space="PSUM") as psA,
            tc.tile_pool(name="psB", bufs=2, space="PSUM") as psB,
        ):
            # ---- persistent tiles ----
            qT = bigpool.tile([P, 2, N], F16, name="qT_sb")        # [pair, n]
            kv = bigpool.tile([P, 16, CH], F16, name="kv_sb")      # per-pair [ke|ko|ve|vo]
            outsc = bigpool.tile([P, 2, N], F16, name="outsc_sb")  # scaled out^T

            # G/W block-diagonal lhsT tiles are zeroed by gpsimd so the PE
            # warmup (opens the HAM clock-gate, results discarded) can start
            # immediately without waiting on any DMA.
            gwG_all = wpool.tile([P, 2, 128], F16, name="gwG_all")
            nc.gpsimd.memset(gwG_all, 0.0)
            gwW_all = wpool.tile([P, 2, 128], F16, name="gwW_all")
            nc.gpsimd.memset(gwW_all, 0.0)
            warm_sb = smallpool.tile([P, 4], F32, name="warm_sb")
            ps_warm = ps4.tile([P, 128], F32, tag="ps4", name="ps_warm")
            for i in range(58):
                nc.tensor.matmul(ps_warm, lhsT=gwG_all[:, 0, :],
                                 rhs=gwW_all[:, 0, :], start=True, stop=True)
            nc.vector.tensor_copy(warm_sb, ps_warm[:, 0:4])

            # Input DMAs all on the sync HWDGE ring, ordered by first use:
            # the fronted q(0) needs wq + x0; the kv phase wkv + x; wo/bo
            # only the back half.  (Splitting rings makes the first-needed
            # tensors share HBM bandwidth with later ones and arrive LATER.)
            wq = wpool.tile([P, KO, 256], F16, name="wq_sb")
            nc.sync.dma_start(wq, wq_d)
            x_all = wpool.tile([P, KO, NCH, CH], F16, name="x_all")
            nc.sync.dma_start(x_all[:, :, 0, :], x_d[0])
            wkv = wpool.tile([P, KO, 512], F16, name="wkv_sb")
            nc.sync.dma_start(wkv, wkv_d)
            for cc in range(1, NCH):
                nc.sync.dma_start(x_all[:, :, cc, :], x_d[cc])
            wo = wpool.tile([P, 2, 1024], F16, name="wo_sb")
            nc.sync.dma_start(wo, wo_d)
            bo = wpool.tile([P, 128], F16, name="bo_sb")
            nc.sync.dma_start(bo, bo_d)

            # ---- helpers (emission order == PE execution order) ----
            prods = {}
            rinv = {}
            ps_o = {}

            def emit_q_mt(c, mt):
                cs = slice(c * CH, (c + 1) * CH)
                ps_q = psA.tile([P, CH], F32, tag="psA", name="ps_q")
                for ko in range(KO):
                    nc.tensor.matmul(
                        ps_q,
                        lhsT=(wq[:, ko, mt * 128:(mt + 1) * 128]),
                        rhs=(x_all[:, ko, c, :]),
                        start=(ko == 0),
                        stop=(ko == KO - 1),
                    )
                if mt == 0:
                    nc.vector.tensor_copy(qT[:, mt, cs], ps_q)
                else:
                    nc.scalar.copy(qT[:, mt, cs], ps_q)

            def emit_B1(c, p):
                # t = G q'; prods = q' * t  (f16-safe thanks to the q/4 scale)
                qs = qT[:, p, c * CH:(c + 1) * CH]
                ps_t = psB.tile([P, CH], F32, tag="psB", name="ps_t")
                nc.tensor.matmul(ps_t, lhsT=(gwG_all[:, p, :]), rhs=(qs),
                                 start=True, stop=True)
                prods[p, c] = stagepool.tile([P, CH], F16, name="prod",
                                             tag="prod", bufs=6)
                nc.vector.tensor_tensor(prods[p, c], ps_t, qs, MULT)

            def emit_B2(c, p):
                # per-head norm2 broadcast via block-identity, then
                # rinv = 1/sqrt(norm2' + (eps/4)^2): Sqrt table (which also
                # holds Copy, so no extra table churn) + fast DVE reciprocal.
                ps_rep = psB.tile([P, CH], F32, tag="psB", name="ps_rep")
                nc.tensor.matmul(ps_rep, lhsT=(bo), rhs=(prods[p, c]),
                                 start=True, stop=True)
                s_t = stagepool.tile([P, CH], F32, name="s_t", tag="s_t",
                                     bufs=4)
                nc.scalar.activation(s_t, ps_rep, AFT.Sqrt, bias=EPS2)
                rinv[p, c] = stagepool.tile([P, CH], F32, name="rinv",
                                            tag="rinv", bufs=6)
                nc.vector.reciprocal_approx_fast(out=rinv[p, c], in_=s_t)

            def emit_O(c, p):
                ps_o[p, c] = psA.tile([P, CH], F32, tag="psA", name="ps_o")
                nc.tensor.matmul(ps_o[p, c], lhsT=(gwW_all[:, p, :]),
                                 rhs=(qT[:, p, c * CH:(c + 1) * CH]),
                                 start=True, stop=True)

            def emit_S(c, p):
                cs = slice(c * CH, (c + 1) * CH)
                nc.vector.tensor_tensor(outsc[:, p, cs], ps_o[p, c],
                                        rinv[p, c], MULT)

            def emit_C(c, j, split_dma=False):
                # w_out partial for out-feature pair (2j, 2j+1).  Stage
                # copies lean on scalar (vector carries the norm chain);
                # split_dma issues per-mt DMAs so the final transfer is
                # small and overlaps the last stage copy.
                cs = slice(c * CH, (c + 1) * CH)
                st = stagepool.tile([P, 2, CH], F16, name="st", tag="st",
                                    bufs=6)
                for t in range(2):
                    mt = 2 * j + t
                    ps_f = ps4.tile([P, CH], F32, tag="ps4", name="ps_f")
                    for kt in range(2):
                        nc.tensor.matmul(
                            ps_f,
                            lhsT=(wo[:, kt, mt * 128:(mt + 1) * 128]),
                            rhs=(outsc[:, kt, cs]),
                            start=(kt == 0),
                            stop=(kt == 1),
                        )
                    if t == 0 and j % 2 == 1:
                        nc.vector.tensor_copy(st[:, t, :], ps_f)
                    else:
                        nc.scalar.copy(st[:, t, :], ps_f)
                    if split_dma:
                        nc.sync.dma_start(out_d[:, mt:mt + 1, cs],
                                          st[:, t:t + 1, :])
                if not split_dma:
                    nc.sync.dma_start(out_d[:, 2 * j:2 * j + 2, cs], st)

            # ---- fronted q(0): its inputs (wq + x0) land first, so it
            # absorbs the DMA fill latency while wkv streams in.
            emit_q_mt(0, 0)
            emit_q_mt(0, 1)

            # elastic filler: keeps the PE busy (and the HAM gate open) if
            # wkv's completion receipt comes back late
            ps_warm2 = ps4.tile([P, 128], F32, tag="ps4", name="ps_warm2")
            for i in range(8):
                nc.tensor.matmul(ps_warm2, lhsT=gwG_all[:, 0, :],
                                 rhs=gwW_all[:, 0, :], start=True, stop=True)
            nc.vector.tensor_copy(warm_sb, ps_warm2[:, 0:4])

            # ---- kv phase: k,v projection + Gram accumulation, all chunks.
            # kv[j, c] = sum_k x^T[k, j] wkv[k, c]   (c = per-pair [k|v] blocks)
            # [G|W] per pair accumulates in a single PSUM group spanning all
            # 16 kv tiles (no intermediate SBUF adds).
            gw_part = [
                psB.tile([P, 256], F32, tag="psB", name=f"gw_part{p}")
                for p in range(2)
            ]
            for c in range(NCH):
                for nt in range(4):
                    jt = c * 4 + nt
                    ps_kv = ps4.tile([P, CH], F32, tag="ps4", name="ps_kv")
                    for ko in range(KO):
                        nc.tensor.matmul(
                            ps_kv,
                            lhsT=(x_all[:, ko, c, nt * 128:(nt + 1) * 128]),
                            rhs=(wkv[:, ko, :]),
                            start=(ko == 0),
                            stop=(ko == KO - 1),
                        )
                    if nt % 2 == 0:
                        nc.scalar.copy(kv[:, jt, :], ps_kv)
                    else:
                        nc.vector.tensor_copy(kv[:, jt, :], ps_kv)
                    for p in range(2):
                        blk = kv[:, jt, p * 256:(p + 1) * 256]
                        nc.tensor.matmul(
                            gw_part[p], lhsT=blk[:, 0:128], rhs=blk,
                            start=(jt == 0), stop=(jt == 15),
                        )

            # block-diagonal lhsT tiles (two heads stacked on K=128),
            # copied straight out of PSUM
            for p in range(2):
                nc.vector.tensor_copy(gwG_all[0:64, p, 0:64], gw_part[p][0:64, 0:64])
                nc.vector.tensor_copy(gwG_all[64:128, p, 64:128], gw_part[p][64:128, 64:128])
                nc.scalar.copy(gwW_all[0:64, p, 0:64], gw_part[p][0:64, 128:192])
                nc.scalar.copy(gwW_all[64:128, p, 64:128], gw_part[p][64:128, 192:256])

            # ---- back half: q projection of chunk c pipelined against the
            # norm chain + w_out projection of earlier chunks.  B1(c) sits at
            # the end of iteration c so prods are ready an iteration ahead;
            # O(k) sits right before S(k) so psA banks free quickly; deferred
            # C pairs fill the drain chains.
            emit_q_mt(1, 0)
            emit_q_mt(1, 1)
            emit_B1(0, 0)
            emit_B1(0, 1)
            emit_B1(1, 0)
            emit_B1(1, 1)

            for c in range(2, NCH):
                k = c - 2
                emit_q_mt(c, 0)
                emit_B2(k, 0)
                emit_B2(k, 1)
                emit_q_mt(c, 1)
                emit_O(k, 0)
                emit_O(k, 1)
                emit_S(k, 0)
                emit_S(k, 1)
                emit_C(k, 0)
                emit_C(k, 1)
                if k > 0:
                    emit_C(k - 1, 2)
                    emit_C(k - 1, 3)
                emit_B1(c, 0)
                emit_B1(c, 1)

            # ---- drain: chunks 2 and 3, with deferred C pairs filling the
            # norm-chain latencies.
            emit_B2(2, 0)
            emit_B2(2, 1)
            emit_O(2, 0)
            emit_O(2, 1)
            emit_C(1, 2)
            emit_C(1, 3)
            emit_S(2, 0)
            emit_S(2, 1)
            emit_C(2, 0)
            emit_C(2, 1)
            emit_B2(3, 0)
            emit_B2(3, 1)
            emit_O(3, 0)
            emit_O(3, 1)
            emit_C(2, 2)
            emit_C(2, 3)
            emit_S(3, 0)
            emit_S(3, 1)
            emit_C(3, 0)
            emit_C(3, 1)
            emit_C(3, 2, split_dma=True)
            emit_C(3, 3, split_dma=True)

    nc.compile()
    return nc


_NC_CACHE = None


def _get_nc():
    global _NC_CACHE
    if _NC_CACHE is None:
        _NC_CACHE = _build_bass()
    return _NC_CACHE


def _build_in_maps(x, w_qkv, w_out_g):
    """Per-core device inputs (shared NEFF, different shards)."""
    bo = np.zeros((P, 128), np.float16)
    bo[0:64, 0:64] = 1.0
    bo[64:128, 64:128] = 1.0

    in_maps = []
    for core in range(NCORES):
        bi = core // 4
        hg = core % 4
        # x^T tiled chunk-major [nch, p, ko, ch]
        xt0 = x[bi].T.reshape(KO, P, N).transpose(1, 0, 2)
        xt = np.ascontiguousarray(
            np.stack([xt0[:, :, cc * CH:(cc + 1) * CH] for cc in range(NCH)]))
        # q rows of this head group, transposed -> [dim, 256] -> [p, ko, 256]
        # scaled by 1/4 (exact): out is homogeneous degree-0 in q, and the
        # scale keeps q'*(Gq') inside fp16 range.
        rows_q = slice(hg * 256, hg * 256 + 256)
        wq = np.ascontiguousarray(
            (w_qkv[rows_q] * 0.25).T.reshape(KO, P, 256).transpose(1, 0, 2))
        # per-head-pair [k_even | k_odd | v_even | v_odd] blocks
        blocks = []
        for pp in range(2):
            he = hg * HPC + 2 * pp
            ho = he + 1
            blocks.append(w_qkv[DIM + he * D: DIM + (he + 1) * D])
            blocks.append(w_qkv[DIM + ho * D: DIM + (ho + 1) * D])
            blocks.append(w_qkv[2 * DIM + he * D: 2 * DIM + (he + 1) * D])
            blocks.append(w_qkv[2 * DIM + ho * D: 2 * DIM + (ho + 1) * D])
        wkv_local = np.concatenate(blocks, axis=0)  # [512, dim]
        wkv = np.ascontiguousarray(
            wkv_local.T.reshape(KO, P, 512).transpose(1, 0, 2))
        # w_out columns for this head group (norm_g folded), transposed
        wo_local = w_out_g[:, hg * 256:(hg + 1) * 256]  # [1024, 256]
        wo = np.ascontiguousarray(
            wo_local.T.reshape(2, P, 1024).transpose(1, 0, 2))
        in_maps.append({
            "xt": xt.astype(np.float16), "wq": wq.astype(np.float16),
            "wkv": wkv.astype(np.float16), "wo": wo.astype(np.float16),
            "bo": bo,
        })
    return in_maps


def kernel(x, w_qkv, w_out, b_out, norm_g, norm_b):
    x = np.ascontiguousarray(np.asarray(x, dtype=np.float32))
    w_qkv = np.asarray(w_qkv, dtype=np.float32)
    w_out = np.asarray(w_out, dtype=np.float32)
    b_out = np.asarray(b_out, dtype=np.float32)
    g = np.asarray(norm_g, dtype=np.float32).reshape(H)
    bb = np.asarray(norm_b, dtype=np.float32).reshape(H)

    # Fold norm_g into w_out columns (attn scale per head passes through @v).
    w_out_g = w_out.copy()
    for h in range(H):
        w_out_g[:, h * D:(h + 1) * D] *= g[h]

    in_maps = _build_in_maps(x, w_qkv, w_out_g)

    nc = _get_nc()
    res = None
    last_exc = None
    for _attempt in range(3):
        try:
            res = run_bass_kernel_spmd(nc, in_maps, core_ids=list(range(NCORES)))
            break
        except Exception as e:  # transient NRT_EXEC_UNIT_UNRECOVERABLE etc.
            last_exc = e
            import time as _time
            _time.sleep(5)
    if res is None:
        raise last_exc

    out = np.zeros((B, N, DIM), np.float32)
    for core in range(NCORES):
        bi = core // 4
        partial = res.results[core]["outT"].reshape(P, 8, N).astype(np.float32)
        out[bi] += partial.transpose(1, 0, 2).reshape(DIM, N).T
    out += b_out[None, None, :]

    # Exact rank-1 correction for norm_b (zero in practice).
    if np.any(bb != 0.0):
        for bi in range(B):
            corr = np.zeros(DIM, np.float64)
            for h in range(H):
                wv = w_qkv[2 * DIM + h * D: 2 * DIM + (h + 1) * D]  # [d, dim]
                vsum = (x[bi].astype(np.float64) @ wv.T.astype(np.float64)).sum(axis=0)
                # the +b term bypasses the g scale, so use the raw w_out
                corr += bb[h] * (w_out[:, h * D:(h + 1) * D].astype(np.float64) @ vsum)
            out[bi] += corr.astype(np.float32)[None, :]

    return out


# revision 19
# speedup vs baseline: 1.0155x; 1.0155x over previous
"""Trainium2 Bass kernel for the L2-normalized attention module.

Reference computation (per batch b):
    qkv = x @ w_qkv.T                        # [n, 3*dim]
    q,k,v per head h (d=64)                  # [n, d]
    dots = q @ k.T                           # [n, n]
    attn = dots / max(||dots_row||_2, eps) * g + b
    out_h = attn @ v                         # [n, d]
    final = concat_h(out_h) @ w_out.T + b_out

Key algebraic factorization used here: the score "nonlinearity" is only a
per-row scale r_i = 1/max(||dots_i||, eps), and ||dots_i||^2 = q_i^T (k^T k) q_i.
Therefore (with W = k^T v, G = k^T k):
    out_h^T = diag-free:  outT[:, i] = r_i * (W^T q_i),   r_i = rsqrt(q_i^T G q_i)
This removes the n x n score matrix entirely (8x fewer FLOPs) and keeps all
on-chip traffic tiny.  Since out is homogeneous degree-0 in q, wq is scaled
by 1/4 on the host (exact power of two) so the q'(Gq') products fit fp16.

Sharding: 8 cores = 2 batches x 4 head-groups (4 heads each).  Each core
computes its qkv slice, the factored attention for its 4 heads, and a partial
w_out projection; the host sums the 4 partials per batch (TP reduction) and
adds b_out.  norm_g is folded into w_out on the host; norm_b (zero in
practice) is handled by an exact host-side rank-1 correction.

Schedule: [warmup matmuls on memset tiles] -> [kv projection + Gram
accumulation for all chunks] -> [software-pipelined back half: q projection
of chunk c overlapped with the norm chain and w_out projection of chunk c-1].
Inputs split across both HWDGE rings (x on sync, weights on scalar).
"""

import numpy as np

from concourse import bacc
import concourse.mybir as mybir
import concourse.tile as tile
from concourse.bass_utils import run_bass_kernel_spmd

# Problem shape (hardcoded per contract)
B, N, DIM, H, D = 2, 2048, 1024, 16, 64
NCORES = 8
HPC = H // 4            # 4 heads per core
CH = 512                # sequence chunk (matmul moving free dim)
NCH = N // CH           # 4
KO = DIM // 128         # 8 contraction tiles for the projections
P = 128

F32 = mybir.dt.float32
F16 = mybir.dt.float16
MULT = mybir.AluOpType.mult
AFT = mybir.ActivationFunctionType
EPS2 = 6.25e-26         # (1e-12 / 4)^2: F.normalize eps with the q/4 scaling


def _build_bass():
    nc = bacc.Bacc("TRN2", target_bir_lowering=False, debug=False)

    # register the Rsqrt bias const (mirrors Bass's const registration)
    _eps_t = nc.alloc_sbuf_tensor("const-float32-epsr", [128, 1], F32)
    nc.gpsimd.memset(_eps_t.ap(), EPS2)
    nc.const_aps.aps[(F32, EPS2)] = _eps_t.ap()

    x_d = nc.dram_tensor("xt", [NCH, P, KO, CH], F16, kind="ExternalInput").ap()
    wq_d = nc.dram_tensor("wq", [P, KO, 256], F16, kind="ExternalInput").ap()
    wkv_d = nc.dram_tensor("wkv", [P, KO, 512], F16, kind="ExternalInput").ap()
    wo_d = nc.dram_tensor("wo", [P, 2, 1024], F16, kind="ExternalInput").ap()
    bo_d = nc.dram_tensor("bo", [P, 128], F16, kind="ExternalInput").ap()
    out_d = nc.dram_tensor("outT", [P, 8, N], F16, kind="ExternalOutput").ap()

    with tile.TileContext(nc) as tc:
        with (
            tc.tile_pool(name="w", bufs=1) as wpool,
            tc.tile_pool(name="big", bufs=1) as bigpool,
            tc.tile_pool(name="small", bufs=4) as smallpool,
            tc.tile_pool(name="stage", bufs=4) as stagepool,
            tc.tile_pool(name="ps4", bufs=4, space="PSUM") as ps4,
            tc.tile_pool(name="psA", bufs=2, space="PSUM") as psA,
            tc.tile_pool(name="psB", bufs=2, space="PSUM") as psB,
        ):
            # ---- persistent tiles ----
            qT = bigpool.tile([P, 2, N], F16, name="qT_sb")        # [pair, n]
            kv = bigpool.tile([P, 16, CH], F16, name="kv_sb")      # per-pair [ke|ko|ve|vo]
            outsc = bigpool.tile([P, 2, N], F16, name="outsc_sb")  # scaled out^T

            # G/W block-diagonal lhsT tiles are zeroed by gpsimd so the PE
            # warmup (opens the HAM clock-gate, results discarded) can start
            # immediately without waiting on any DMA.
            gwG_all = wpool.tile([P, 2, 128], F16, name="gwG_all")
            nc.gpsimd.memset(gwG_all, 0.0)
            gwW_all = wpool.tile([P, 2, 128], F16, name="gwW_all")
            nc.gpsimd.memset(gwW_all, 0.0)
            warm_sb = smallpool.tile([P, 4], F32, name="warm_sb")
            ps_warm = ps4.tile([P, 128], F32, tag="ps4", name="ps_warm")
            for i in range(58):
                nc.tensor.matmul(ps_warm, lhsT=gwG_all[:, 0, :],
                                 rhs=gwW_all[:, 0, :], start=True, stop=True)
            nc.vector.tensor_copy(warm_sb, ps_warm[:, 0:4])

            # Input DMAs all on the sync HWDGE ring, ordered by first use:
            # the fronted q(0) needs wq + x0; the kv phase wkv + x; wo/bo
            # only the back half.  (Splitting rings makes the first-needed
            # tensors share HBM bandwidth with later ones and arrive LATER.)
            wq = wpool.tile([P, KO, 256], F16, name="wq_sb")
            nc.sync.dma_start(wq, wq_d)
            x_all = wpool.tile([P, KO, NCH, CH], F16, name="x_all")
            nc.sync.dma_start(x_all[:, :, 0, :], x_d[0])
            wkv = wpool.tile([P, KO, 512], F16, name="wkv_sb")
            nc.sync.dma_start(wkv, wkv_d)
            for cc in range(1, NCH):
                nc.sync.dma_start(x_all[:, :, cc, :], x_d[cc])
            wo = wpool.tile([P, 2, 1024], F16, name="wo_sb")
            nc.sync.dma_start(wo, wo_d)
            bo = wpool.tile([P, 128], F16, name="bo_sb")
            nc.sync.dma_start(bo, bo_d)

            # ---- helpers (emission order == PE execution order) ----
            prods = {}
            rinv = {}
            ps_o = {}

            def emit_q_mt(c, mt):
                cs = slice(c * CH, (c + 1) * CH)
                ps_q = psA.tile([P, CH], F32, tag="psA", name="ps_q")
                for ko in range(KO):
                    nc.tensor.matmul(
                        ps_q,
                        lhsT=(wq[:, ko, mt * 128:(mt + 1) * 128]),
                        rhs=(x_all[:, ko, c, :]),
                        start=(ko == 0),
                        stop=(ko == KO - 1),
                    )
                if mt == 0:
                    nc.vector.tensor_copy(qT[:, mt, cs], ps_q)
                else:
                    nc.scalar.copy(qT[:, mt, cs], ps_q)

            def emit_B1(c, p):
                # t = G q'; prods = q' * t  (f16-safe thanks to the q/4 scale)
                qs = qT[:, p, c * CH:(c + 1) * CH]
                ps_t = psB.tile([P, CH], F32, tag="psB", name="ps_t")
                nc.tensor.matmul(ps_t, lhsT=(gwG_all[:, p, :]), rhs=(qs),
                                 start=True, stop=True)
                prods[p, c] = stagepool.tile([P, CH], F16, name="prod",
                                             tag="prod", bufs=6)
                nc.vector.tensor_tensor(prods[p, c], ps_t, qs, MULT)

            def emit_B2(c, p):
                # per-head norm2 broadcast via block-identity, then
                # rinv = 1/sqrt(norm2' + (eps/4)^2): Sqrt table (which also
                # holds Copy, so no extra table churn) + fast DVE reciprocal.
                ps_rep = psB.tile([P, CH], F32, tag="psB", name="ps_rep")
                nc.tensor.matmul(ps_rep, lhsT=(bo), rhs=(prods[p, c]),
                                 start=True, stop=True)
                s_t = stagepool.tile([P, CH], F32, name="s_t", tag="s_t",
                                     bufs=4)
                nc.scalar.activation(s_t, ps_rep, AFT.Sqrt, bias=EPS2)
                rinv[p, c] = stagepool.tile([P, CH], F32, name="rinv",
                                            tag="rinv", bufs=6)
                nc.vector.reciprocal_approx_fast(out=rinv[p, c], in_=s_t)

            def emit_O(c, p):
                ps_o[p, c] = psA.tile([P, CH], F32, tag="psA", name="ps_o")
                nc.tensor.matmul(ps_o[p, c], lhsT=(gwW_all[:, p, :]),
                                 rhs=(qT[:, p, c * CH:(c + 1) * CH]),
                                 start=True, stop=True)

            def emit_S(c, p):
                cs = slice(c * CH, (c + 1) * CH)
                nc.vector.tensor_tensor(outsc[:, p, cs], ps_o[p, c],
                                        rinv[p, c], MULT)

            def emit_C(c, j, engines=None):
                # w_out partial for out-feature pair (2j, 2j+1), one DMA.
                # Stage copies lean on scalar by default (vector carries the
                # norm chain); `engines` overrides per-mt engine choice.
                if engines is None:
                    engines = ("v", "s") if j % 2 == 1 else ("s", "s")
                cs = slice(c * CH, (c + 1) * CH)
                st = stagepool.tile([P, 2, CH], F16, name="st", tag="st",
                                    bufs=6)
                for t in range(2):
                    mt = 2 * j + t
                    ps_f = ps4.tile([P, CH], F32, tag="ps4", name="ps_f")
                    for kt in range(2):
                        nc.tensor.matmul(
                            ps_f,
                            lhsT=(wo[:, kt, mt * 128:(mt + 1) * 128]),
                            rhs=(outsc[:, kt, cs]),
                            start=(kt == 0),
                            stop=(kt == 1),
                        )
                    if engines[t] == "v":
                        nc.vector.tensor_copy(st[:, t, :], ps_f)
                    else:
                        nc.scalar.copy(st[:, t, :], ps_f)
                nc.sync.dma_start(out_d[:, 2 * j:2 * j + 2, cs], st)

            # ---- fronted q(0): its inputs (wq + x0) land first, so it
            # absorbs the DMA fill latency while wkv streams in.
            emit_q_mt(0, 0)
            emit_q_mt(0, 1)

            # elastic filler: keeps the PE busy (and the HAM gate open) if
            # wkv's completion receipt comes back late
            ps_warm2 = ps4.tile([P, 128], F32, tag="ps4", name="ps_warm2")
            for i in range(8):
                nc.tensor.matmul(ps_warm2, lhsT=gwG_all[:, 0, :],
                                 rhs=gwW_all[:, 0, :], start=True, stop=True)
            nc.vector.tensor_copy(warm_sb, ps_warm2[:, 0:4])

            # ---- kv phase: k,v projection + Gram accumulation, all chunks.
            # kv[j, c] = sum_k x^T[k, j] wkv[k, c]   (c = per-pair [k|v] blocks)
            # [G|W] per pair accumulates in a single PSUM group spanning all
            # 16 kv tiles (no intermediate SBUF adds).
            gw_part = [
                psB.tile([P, 256], F32, tag="psB", name=f"gw_part{p}")
                for p in range(2)
            ]
            for c in range(NCH):
                for nt in range(4):
                    jt = c * 4 + nt
                    ps_kv = ps4.tile([P, CH], F32, tag="ps4", name="ps_kv")
                    for ko in range(KO):
                        nc.tensor.matmul(
                            ps_kv,
                            lhsT=(x_all[:, ko, c, nt * 128:(nt + 1) * 128]),
                            rhs=(wkv[:, ko, :]),
                            start=(ko == 0),
                            stop=(ko == KO - 1),
                        )
                    if nt % 2 == 0:
                        nc.scalar.copy(kv[:, jt, :], ps_kv)
                    else:
                        nc.vector.tensor_copy(kv[:, jt, :], ps_kv)
                    for p in range(2):
                        blk = kv[:, jt, p * 256:(p + 1) * 256]
                        nc.tensor.matmul(
                            gw_part[p], lhsT=blk[:, 0:128], rhs=blk,
                            start=(jt == 0), stop=(jt == 15),
                        )

            # block-diagonal lhsT tiles (two heads stacked on K=128),
            # copied straight out of PSUM
            for p in range(2):
                nc.vector.tensor_copy(gwG_all[0:64, p, 0:64], gw_part[p][0:64, 0:64])
                nc.vector.tensor_copy(gwG_all[64:128, p, 64:128], gw_part[p][64:128, 64:128])
                nc.scalar.copy(gwW_all[0:64, p, 0:64], gw_part[p][0:64, 128:192])
                nc.scalar.copy(gwW_all[64:128, p, 64:128], gw_part[p][64:128, 192:256])

            # ---- back half: q projection of chunk c pipelined against the
            # norm chain + w_out projection of earlier chunks.  B1(c) sits at
            # the end of iteration c so prods are ready an iteration ahead;
            # O(k) sits right before S(k) so psA banks free quickly; deferred
            # C pairs fill the drain chains.
            emit_q_mt(1, 0)
            emit_q_mt(1, 1)
            emit_B1(0, 0)
            emit_B1(0, 1)
            emit_B1(1, 0)
            emit_B1(1, 1)

            for c in range(2, NCH):
                k = c - 2
                emit_q_mt(c, 0)
                emit_B2(k, 0)
                emit_B2(k, 1)
                emit_q_mt(c, 1)
                emit_O(k, 0)
                emit_O(k, 1)
                emit_S(k, 0)
                emit_S(k, 1)
                emit_C(k, 0)
                emit_C(k, 1)
                if k > 0:
                    emit_C(k - 1, 2)
                    emit_C(k - 1, 3)
                emit_B1(c, 0)
                emit_B1(c, 1)

            # ---- drain: chunks 2 and 3, with deferred C pairs filling the
            # norm-chain latencies.
            emit_B2(2, 0)
            emit_B2(2, 1)
            emit_O(2, 0)
            emit_O(2, 1)
            emit_C(1, 2)
            emit_C(1, 3)
            emit_S(2, 0)
            emit_S(2, 1)
            emit_C(2, 0)
            emit_C(2, 1)
            emit_B2(3, 0)
            emit_B2(3, 1)
            emit_O(3, 0)
            emit_O(3, 1)
            emit_C(2, 2)
            emit_C(2, 3)
            emit_S(3, 0)
            emit_S(3, 1)
            emit_C(3, 0)
            emit_C(3, 1)
            emit_C(3, 2, engines=("s", "v"))
            emit_C(3, 3, engines=("v", "s"))

    nc.compile()
    return nc


_NC_CACHE = None


def _get_nc():
    global _NC_CACHE
    if _NC_CACHE is None:
        _NC_CACHE = _build_bass()
    return _NC_CACHE


def _build_in_maps(x, w_qkv, w_out_g):
    """Per-core device inputs (shared NEFF, different shards)."""
    bo = np.zeros((P, 128), np.float16)
    bo[0:64, 0:64] = 1.0
    bo[64:128, 64:128] = 1.0

    in_maps = []
    for core in range(NCORES):
        bi = core // 4
        hg = core % 4
        # x^T tiled chunk-major [nch, p, ko, ch]
        xt0 = x[bi].T.reshape(KO, P, N).transpose(1, 0, 2)
        xt = np.ascontiguousarray(
            np.stack([xt0[:, :, cc * CH:(cc + 1) * CH] for cc in range(NCH)]))
        # q rows of this head group, transposed -> [dim, 256] -> [p, ko, 256]
        # scaled by 1/4 (exact): out is homogeneous degree-0 in q, and the
        # scale keeps q'*(Gq') inside fp16 range.
        rows_q = slice(hg * 256, hg * 256 + 256)
        wq = np.ascontiguousarray(
            (w_qkv[rows_q] * 0.25).T.reshape(KO, P, 256).transpose(1, 0, 2))
        # per-head-pair [k_even | k_odd | v_even | v_odd] blocks
        blocks = []
        for pp in range(2):
            he = hg * HPC + 2 * pp
            ho = he + 1
            blocks.append(w_qkv[DIM + he * D: DIM + (he + 1) * D])
            blocks.append(w_qkv[DIM + ho * D: DIM + (ho + 1) * D])
            blocks.append(w_qkv[2 * DIM + he * D: 2 * DIM + (he + 1) * D])
            blocks.append(w_qkv[2 * DIM + ho * D: 2 * DIM + (ho + 1) * D])
        wkv_local = np.concatenate(blocks, axis=0)  # [512, dim]
        wkv = np.ascontiguousarray(
            wkv_local.T.reshape(KO, P, 512).transpose(1, 0, 2))
        # w_out columns for this head group (norm_g folded), transposed
        wo_local = w_out_g[:, hg * 256:(hg + 1) * 256]  # [1024, 256]
        wo = np.ascontiguousarray(
            wo_local.T.reshape(2, P, 1024).transpose(1, 0, 2))
        in_maps.append({
            "xt": xt.astype(np.float16), "wq": wq.astype(np.float16),
            "wkv": wkv.astype(np.float16), "wo": wo.astype(np.float16),
            "bo": bo,
        })
    return in_maps


def kernel(x, w_qkv, w_out, b_out, norm_g, norm_b):
    x = np.ascontiguousarray(np.asarray(x, dtype=np.float32))
    w_qkv = np.asarray(w_qkv, dtype=np.float32)
    w_out = np.asarray(w_out, dtype=np.float32)
    b_out = np.asarray(b_out, dtype=np.float32)
    g = np.asarray(norm_g, dtype=np.float32).reshape(H)
    bb = np.asarray(norm_b, dtype=np.float32).reshape(H)

    # Fold norm_g into w_out columns (attn scale per head passes through @v).
    w_out_g = w_out.copy()
    for h in range(H):
        w_out_g[:, h * D:(h + 1) * D] *= g[h]

    in_maps = _build_in_maps(x, w_qkv, w_out_g)

    nc = _get_nc()
    res = None
    last_exc = None
    for _attempt in range(3):
        try:
            res = run_bass_kernel_spmd(nc, in_maps, core_ids=list(range(NCORES)))
            break
        except Exception as e:  # transient NRT_EXEC_UNIT_UNRECOVERABLE etc.
            last_exc = e
            import time as _time
            _time.sleep(5)
    if res is None:
        raise last_exc

    out = np.zeros((B, N, DIM), np.float32)
    for core in range(NCORES):
        bi = core // 4
        partial = res.results[core]["outT"].reshape(P, 8, N).astype(np.float32)
        out[bi] += partial.transpose(1, 0, 2).reshape(DIM, N).T
    out += b_out[None, None, :]

    # Exact rank-1 correction for norm_b (zero in practice).
    if np.any(bb != 0.0):
        for bi in range(B):
            corr = np.zeros(DIM, np.float64)
            for h in range(H):
                wv = w_qkv[2 * DIM + h * D: 2 * DIM + (h + 1) * D]  # [d, dim]
                vsum = (x[bi].astype(np.float64) @ wv.T.astype(np.float64)).sum(axis=0)
                # the +b term bypasses the g scale, so use the raw w_out
                corr += bb[h] * (w_out[:, h * D:(h + 1) * D].astype(np.float64) @ vsum)
            out[bi] += corr.astype(np.float32)[None, :]

    return out
